# revision 38
# baseline (speedup 1.0000x reference)
"""Causal multi-head self-attention on 8 Trainium2 NeuronCores.

Problem: x[4, 2048, 2048] fp32, w_q/w_k/w_v/w_o [2048, 2048] fp32,
16 heads x d_head=128. out = softmax(causal(QK^T/sqrt(128))) V @ w_o.

Sharding: tensor-parallel over heads. Core c owns heads {2c, 2c+1}:
computes Q^T/K^T ([d_head, tokens]) and V ([tokens, d_head]) for its
heads from the full x (streamed pre-transposed as x^T in bf16), runs
per-head causal attention producing attnT [d_head, tokens], then the
partial output o_c = attn_c @ w_o[head rows] in bf16; the host sums
the 8 partials.

All matmuls are bf16 (fp32 PSUM accumulation): full PE rate at any
moving width, FWL weight loads. Softmax skips max-subtraction (scores
are O(+-6); exp is safe): probsT = exp(scale*scoresT) is computed in
[k, q] orientation (K^T stationary) so no transposes are needed.
Causal masking: exact 128-wide chunk narrowing, with a gpsimd
affine_select only on the 128x128 triangle subtile. Row sums are
accumulated in bf16 by the DVE and reduced over partitions with a
ones-vector matmul, broadcast across partitions with a rank-1 f32r
matmul, and inverted with the DVE's fast approximate reciprocal
straight out of PSUM. The output projection is software-pipelined one
q-superblock behind the attention (so the normalization chain never
blocks the PE queue); its PSUM->SBUF copies ride the vector engine
(whose queue congestion is harmless there), staged in a [128, 2048]
bf16 tile and stored with one DMA per 128 tokens. Weight preloads ride
the scalar engine's HWDGE queue so the x stream leads the sync queue.
PSUM banks: 3 proj/norm (shared tag) + 3 scores/O-proj (shared tag)
+ 2 attn-accumulator. Phases are kept separated (projections | attention+O) so
each phase's helper-engine copies land in windows where those queues
are idle — interleaving them measurably loses (v7/v9 experiments).
"""

import contextlib

import ml_dtypes
import numpy as np

import concourse.bass as bass
from concourse import bacc
import concourse.mybir as mybir
from concourse.tile import TileContext
from concourse.bass_utils import run_bass_kernel_spmd

B = 4
S = 2048
D = 2048
NH = 16
DH = 128
N_CORES = 8
HPC = NH // N_CORES          # heads per core = 2
HD = HPC * DH                # head dims per core = 256
KO = D // 128                # contraction chunks = 16
TSB = 512                    # projection token superblock
NSB = S // TSB               # 4
QSB = 512                    # attention q superblock
NQSB = S // QSB              # 4
NKC = S // 128               # 16 k-chunks per batch
SCALE = float(1.0 / np.sqrt(DH))

F32 = mybir.dt.float32
F32R = mybir.dt.float32r
BF = mybir.dt.bfloat16

_CACHED = {}


def build(loop_r: int | None = None):
    nc = bacc.Bacc("TRN2", target_bir_lowering=False, debug=False)
    xT = nc.dram_tensor("xT", [B, D, S], BF, kind="ExternalInput")
    wq = nc.dram_tensor("wq", [D, HD], BF, kind="ExternalInput")
    wk = nc.dram_tensor("wk", [D, HD], BF, kind="ExternalInput")
    wv = nc.dram_tensor("wv", [D, HD], BF, kind="ExternalInput")
    wo = nc.dram_tensor("wo", [HD, D], BF, kind="ExternalInput")
    out = nc.dram_tensor("out", [B, S, D], BF, kind="ExternalOutput")

    env = {
        "xTr": xT.ap().rearrange("b (ko p) s -> b p ko s", p=128),
        "out": out,
    }

    with TileContext(nc) as tc:
        with tc.tile_pool(name="const", bufs=1) as const, \
             tc.tile_pool(name="big", bufs=1) as big, \
             tc.tile_pool(name="vpool", bufs=2) as vpool, \
             tc.tile_pool(name="stream", bufs=2) as stream, \
             tc.tile_pool(name="work", bufs=4) as work, \
             tc.tile_pool(name="ps", bufs=2, space="PSUM") as ps:

            # ---- weights / constants (resident) ----
            wq_sb = const.tile([128, KO, HD], BF)
            wk_sb = const.tile([128, KO, HD], BF)
            wv_sb = const.tile([128, KO, HD], BF)
            wo_sb = const.tile([128, HPC, D], BF)
            # all weights ride the scalar engine's HWDGE queue (idle at
            # startup) in quarter-granularity DMAs, so the x stream leads
            # the sync queue and both queues fill SBUF in parallel.
            for (wt, wsb) in ((wq, wq_sb), (wk, wk_sb), (wv, wv_sb)):
                wv_ = wt.ap().rearrange("(ko p) m -> p ko m", p=128)
                for kq in range(4):
                    nc.scalar.dma_start(wsb[:, kq * 4:(kq + 1) * 4],
                                        wv_[:, kq * 4:(kq + 1) * 4])
            nc.scalar.dma_start(
                wo_sb, wo.ap().rearrange("(c p) n -> p c n", p=128))
            ones_col = const.tile([128, 1], BF)
            ones_row = const.tile([1, 128], F32R)
            tmp1 = const.tile([128, 1], F32)
            nc.vector.memset(tmp1, 1.0)
            nc.vector.tensor_copy(ones_col, tmp1)
            nc.vector.tensor_copy(ones_row, tmp1[0:1, 0:1].to_broadcast([1, 128]))

            env.update(wq_sb=wq_sb, wk_sb=wk_sb, wv_sb=wv_sb, wo_sb=wo_sb,
                       ones_col=ones_col, ones_row=ones_row,
                       big=big, vpool=vpool, stream=stream, work=work, ps=ps)

            loop_cm = (tc.For_i(0, loop_r, 1) if loop_r else
                       contextlib.nullcontext())
            with loop_cm:
                _batches(nc, env)

    nc.compile()
    return nc


def _batches(nc, env):
    xTr = env["xTr"]
    big, vpool, stream = env["big"], env["vpool"], env["stream"]

    # qt is written/read at matching subtile granularity across batches, so
    # one resident tile overlaps batches via subtile WAR deps. kt/v regions
    # are read until the very end of a batch's attention, so they rotate
    # through 2 slots instead.
    qt_sb = big.tile([128, HPC, S], BF, tag="qt", bufs=1)

    def load_xt(b, s):
        xt = stream.tile([128, KO, TSB], BF, tag="xt", bufs=2)
        # the very first block arrives in quarters so the first matmuls
        # can start ~4x sooner; later blocks use halves (fewer DMAs)
        nq = 4 if (b, s) == (0, 0) else 2
        for kq in range(nq):
            w = KO // nq
            nc.sync.dma_start(
                xt[:, kq * w:(kq + 1) * w],
                xTr[b, :, kq * w:(kq + 1) * w, s * TSB:(s + 1) * TSB])
        return xt

    pending = {}
    for b in range(B):
        # ===== projections (phase-separated: their PSUM copies run while
        # the scalar/vector queues are otherwise idle) =====
        kt_sb = vpool.tile([128, HPC, S], BF, tag="kt", bufs=2)
        v_sb = vpool.tile([128, NKC, HD], BF, tag="v", bufs=2)
        for s in range(NSB):
            xt = pending.pop((b, s), None)
            if xt is None:
                xt = load_xt(b, s)
            _proj_s(nc, env, qt_sb, kt_sb, v_sb, s, xt)

        # prefetch the next batch's first two x superblocks now, ahead of
        # this batch's output stores in the sync-queue FIFO (their slots'
        # projection reads are already done; the stores wait ~2us each on
        # PSUM copies and would delay the loads past the batch boundary)
        if b + 1 < B:
            pending[(b + 1, 0)] = load_xt(b + 1, 0)
            pending[(b + 1, 1)] = load_xt(b + 1, 1)

        # ===== attention + fused output projection, per q-superblock =====
        # The O-projection is software-pipelined one superblock behind the
        # attention so the normalization chain (sum -> f32r copy -> bcast
        # -> reciprocal -> scale) never blocks the PE queue: O(j-1) only
        # issues after attention(j), by which time at(j-1) is long ready.
        prev_at = None
        for j in range(NQSB):
            cur_at = _attn_j(nc, env, qt_sb, kt_sb, v_sb, j)
            if prev_at is not None:
                _o_proj(nc, env, b, j - 1, prev_at)
            prev_at = cur_at
        _o_proj(nc, env, b, NQSB - 1, prev_at)


def _proj_s(nc, env, qt_sb, kt_sb, v_sb, s, xt):
    """Projections for one 512-token s-block: Q^T/K^T (weights
    stationary, tokens moving) and V (x^T chunks stationary)."""
    wq_sb, wk_sb, wv_sb = env["wq_sb"], env["wk_sb"], env["wv_sb"]
    ps = env["ps"]
    for m in range(HPC):
        psq = ps.tile([128, TSB], F32, tag="psProj", bufs=3)
        for ko in range(KO):
            nc.tensor.matmul(
                psq, wq_sb[:, ko, m * 128:(m + 1) * 128], xt[:, ko],
                start=(ko == 0), stop=(ko == KO - 1))
        nc.scalar.copy(qt_sb[:, m, s * TSB:(s + 1) * TSB], psq)
        psk = ps.tile([128, TSB], F32, tag="psProj", bufs=3)
        for ko in range(KO):
            nc.tensor.matmul(
                psk, wk_sb[:, ko, m * 128:(m + 1) * 128], xt[:, ko],
                start=(ko == 0), stop=(ko == KO - 1))
        nc.scalar.copy(kt_sb[:, m, s * TSB:(s + 1) * TSB], psk)
    for t in range(TSB // 128):
        tc_idx = s * (TSB // 128) + t
        psv = ps.tile([128, HD], F32, tag="psProj", bufs=3)
        for ko in range(KO):
            nc.tensor.matmul(
                psv, xt[:, ko, t * 128:(t + 1) * 128], wv_sb[:, ko, :],
                start=(ko == 0), stop=(ko == KO - 1))
        nc.vector.tensor_copy(v_sb[:, tc_idx, :], psv)


def _attn_j(nc, env, qt_sb, kt_sb, v_sb, j):
    """One q-superblock of attention. Scores are emitted 2 chunks ahead
    of their PV (skew) so the PE queue never blocks on the exp chain.
    The normalization tail is PE-free: partition-reduce via ones matmul,
    fast approximate reciprocal (~51 ULP) on the DVE straight out of
    PSUM, partition-broadcast on the gpsimd engine, scale on the DVE —
    so however late the denominator resolves, the PE never waits."""
    ones_col, ones_row = env["ones_col"], env["ones_row"]
    work, ps = env["work"], env["ps"]
    nkc = 4 * (j + 1)
    at_tiles = []
    SKEW = 3  # matches the 3 psSO slots / 3 pt bufs; the extra chunk of
    #           slack covers the gpsimd triangle-select latency on
    #           diagonal chunks that skew-2 left exposed
    for h in range(HPC):
        acc = work.tile([128, QSB], BF, tag="acc", bufs=2)
        ps_at = ps.tile([128, QSB], F32, tag="psAT", bufs=2)
        pts = {}
        for i in range(nkc + SKEW):
            if i < nkc:
                c = i
                # exact causal narrowing: on diagonal chunks only
                # q-cols >= c*128 matter (bf16 runs full rate at any
                # moving width); the 128-wide triangle subtile is the
                # only region needing a mask.
                off = max(0, c * 128 - j * QSB)
                ps_s = ps.tile([128, QSB], F32, tag="psSO", bufs=3)
                nc.tensor.matmul(
                    ps_s[:, off:], kt_sb[:, h, c * 128:(c + 1) * 128],
                    qt_sb[:, h, j * QSB + off:(j + 1) * QSB],
                    start=True, stop=True)
                pt = work.tile([128, QSB], BF, tag="pt", bufs=3)
                nc.scalar.activation(
                    pt[:, off:], ps_s[:, off:],
                    mybir.ActivationFunctionType.Exp, scale=SCALE)
                if c >= 4 * j:
                    # causal: keep q - k >= 0 on the 128-wide triangle
                    nc.gpsimd.affine_select(
                        out=pt[:, off:off + 128], in_=pt[:, off:off + 128],
                        compare_op=mybir.AluOpType.is_ge,
                        fill=0.0,
                        base=0,
                        pattern=[[1, 128]],
                        channel_multiplier=-1)
                pts[c] = pt
            if i >= SKEW:
                c = i - SKEW
                off = max(0, c * 128 - j * QSB)
                pt = pts.pop(c)
                if c == 0:
                    nc.vector.tensor_copy(acc, pt)
                else:
                    nc.vector.tensor_add(
                        acc[:, off:], acc[:, off:], pt[:, off:])
                nc.tensor.matmul(
                    ps_at[:, off:], v_sb[:, c, h * 128:(h + 1) * 128],
                    pt[:, off:],
                    start=(c == 0), stop=(c == nkc - 1))
        # norm tiles share the psProj tag (idle during attention), which
        # frees a bank to give the projection groups a third buffer
        psn = ps.tile([128, QSB], F32, tag="psProj", bufs=3, name="psn")
        nc.tensor.matmul(psn[0:1, :], ones_col, acc, start=True, stop=True)
        den_r = work.tile([1, QSB], F32R, tag="denr", bufs=2)
        with nc.allow_low_precision(
                reason="f32r denominator: 2^-14 rounding is fine"):
            nc.vector.tensor_copy(den_r, psn[0:1, :])
        psb = ps.tile([128, QSB], F32, tag="psProj", bufs=3, name="psb")
        nc.tensor.matmul(psb, ones_row, den_r, start=True, stop=True)
        inv_bc = work.tile([128, QSB], F32, tag="invbc", bufs=2)
        nc.vector.reciprocal_approx_fast(
            out=inv_bc[:, :], in_=psb[:, :])
        at = work.tile([128, QSB], BF, tag="at", bufs=4)
        nc.vector.tensor_mul(at, ps_at, inv_bc)
        at_tiles.append(at)
    return at_tiles


def _o_proj(nc, env, b, j, at_tiles):
    """O-projection for superblock j. Output copies all go to the DVE:
    its queue congestion is harmless (nothing PE-critical follows it),
    while scalar-queue congestion would delay the next superblock's
    exps."""
    wo_sb, out = env["wo_sb"], env["out"]
    work, ps = env["work"], env["ps"]
    for t in range(QSB // 128):
        o_st = work.tile([128, D], BF, tag="ost", bufs=2)
        for n in range(D // 512):
            ps_o = ps.tile([128, 512], F32, tag="psSO", bufs=3)
            for h in range(HPC):
                nc.tensor.matmul(
                    ps_o, at_tiles[h][:, t * 128:(t + 1) * 128],
                    wo_sb[:, h, n * 512:(n + 1) * 512],
                    start=(h == 0), stop=(h == HPC - 1))
            nc.vector.tensor_copy(o_st[:, n * 512:(n + 1) * 512], ps_o)
        nc.sync.dma_start(
            out.ap()[b, j * QSB + t * 128:j * QSB + (t + 1) * 128, :],
            o_st)


def kernel(x, w_q, w_k, w_v, w_o, _trace=False):
    bf = ml_dtypes.bfloat16
    x = np.asarray(x, dtype=np.float32)
    xT = np.ascontiguousarray(x.transpose(0, 2, 1)).astype(bf)
    in_maps = []
    for c in range(N_CORES):
        sl = slice(c * HD, (c + 1) * HD)
        in_maps.append({
            "xT": xT,
            "wq": np.ascontiguousarray(np.asarray(w_q, np.float32)[:, sl]).astype(bf),
            "wk": np.ascontiguousarray(np.asarray(w_k, np.float32)[:, sl]).astype(bf),
            "wv": np.ascontiguousarray(np.asarray(w_v, np.float32)[:, sl]).astype(bf),
            "wo": np.ascontiguousarray(np.asarray(w_o, np.float32)[sl, :]).astype(bf),
        })
    if "nc" not in _CACHED:
        _CACHED["nc"] = build()
    res = run_bass_kernel_spmd(
        _CACHED["nc"], in_maps, core_ids=list(range(N_CORES)),
        trace=_trace)
    if _trace:
        _CACHED["last_result"] = res
    acc = np.zeros((B, S, D), dtype=np.float64)
    for r in res.results:
        acc += np.asarray(r["out"], dtype=np.float32)
    return acc.astype(np.float32)
